# revision 31
# baseline (speedup 1.0000x reference)
"""GSNConv (GIN message passing) Bass kernel for Trainium2, 8 NeuronCores.

Strategy (dst-sharded, gather-based, v3):
  - Nodes sharded by destination across 8 cores (12544 per core, 98 blocks
    of 128 = "windows").
  - node_attr stored as a bf16 PAIR table: row k = concat(x[2k], x[2k+1])
    (256 B rows satisfy dma_gather's elem-size constraint at bf16). Each
    edge gathers its src pair; the correct half is selected by a static
    column slice chosen via the edge's src parity.
  - Per core, edges sorted by (supergroup of 7 windows, src chunk, window,
    parity). One dma_gather per (supergroup, chunk), call lengths unified
    across cores, padded to 128 with zeros-row gathers; calls round-robin
    4 SWDGE queues (descriptor generation on gpsimd is the critical path).
  - No per-bucket padding: tiles may straddle (window, parity) boundaries;
    straddling tiles get matmuls for each bucket they touch, with one-hots
    built from block-shifted iota vs in-supergroup dst position (is_equal).
  - Aggregation matmuls and the 2-layer MLP run in bf16 (fp32 PSUM
    accumulation); fp32 is 4x slower on the PE.
"""

from contextlib import ExitStack

import numpy as np

import concourse.bass as bass
import concourse.tile as tile
from concourse import bass_utils, mybir
from concourse._compat import with_exitstack

# Problem shape (hardcoded per contract).
N = 100000
E = 1600000
D = 64
DH = 128
P = 128

NCORES = 8
NPC = 12544               # nodes per core
NWIN = NPC // P           # 98 windows (= output blocks) per core
SGB = 7                   # windows (=blocks) per supergroup
NSG = NWIN // SGB         # 14 supergroups
CH = 25000                # src chunk rows
NPAIR = CH // 2           # 12500 pairs per chunk (zeros pair at NPAIR)
NCHUNK = 4
TAB_ROWS = NCHUNK * (NPAIR + 1)  # pair-table rows
NQ = 4                    # SWDGE queues


def _plan(edge_src: np.ndarray, edge_dst: np.ndarray):
    """Sort/bucket edges; build per-core int16-wrapped pair-idx + dstw arrays,
    unified call lengths and per-(call, window, parity) tile spans."""
    src = edge_src.astype(np.int64)
    dst = edge_dst.astype(np.int64)

    core = dst // NPC
    ldst = dst - core * NPC
    w = ldst // P
    sg = w // SGB
    b_local = w - sg * SGB
    chunk = src // CH
    sloc = src - chunk * CH
    pidx = sloc // 2
    par = sloc & 1
    # in-supergroup dst position, parity-tagged so straddling tiles cannot
    # match the other parity's bucket of the same window
    dpos = (128 * SGB) * par + 128 * b_local + (ldst - w * P)

    # sort edges by (core, sg, chunk, w, parity)
    key = ((((core * NSG + sg) * NCHUNK + chunk) * NWIN + w) * 2 + par).astype(np.int64)
    order = np.argsort(key, kind="stable")
    key_s = key[order]
    pidx_s = pidx[order]
    dpos_s = dpos[order]

    nb = NWIN * 2                                             # (w, parity) buckets
    ngroups = NCORES * NSG * NCHUNK * nb
    counts = np.bincount(key_s, minlength=ngroups).reshape(NCORES, NSG, NCHUNK, nb)
    call_core = counts.sum(axis=3)                            # [NCORES, NSG, NCHUNK]
    ucall = call_core.max(axis=0)
    ucall = ((ucall + P - 1) // P) * P
    ucall = np.maximum(ucall, P)

    call_base = np.zeros((NSG, NCHUNK), np.int64)
    pos = 0
    for s in range(NSG):
        for c in range(NCHUNK):
            call_base[s, c] = pos
            pos += ucall[s, c]
    cap_total = int(pos)
    cols = cap_total // P

    # per-core bucket starts within the call
    bstart = np.zeros((NCORES, NSG, NCHUNK, nb), np.int64)
    np.cumsum(counts[:, :, :, :-1], axis=3, out=bstart[:, :, :, 1:])
    bend = bstart + counts

    # per-edge slot position
    starts = np.zeros(ngroups + 1, np.int64)
    np.cumsum(np.bincount(key_s, minlength=ngroups), out=starts[1:])
    rank = np.arange(len(key_s)) - starts[key_s]
    core_s = key_s // (NSG * NCHUNK * nb)
    rem = key_s - core_s * (NSG * NCHUNK * nb)
    sg_s = rem // (NCHUNK * nb)
    rem2 = rem - sg_s * (NCHUNK * nb)
    c_s = rem2 // nb
    b_s = rem2 - c_s * nb
    pos_edge = call_base[sg_s, c_s] + bstart[core_s, sg_s, c_s, b_s] + rank

    slot_idx = np.full((NCORES, cap_total), NPAIR, np.int16)    # pad -> zeros pair
    slot_dpos = np.full((NCORES, cap_total), -1.0, np.float32)  # pad -> no window
    slot_idx[core_s, pos_edge] = pidx_s.astype(np.int16)
    slot_dpos[core_s, pos_edge] = dpos_s.astype(np.float32)

    # per-(sg, chunk, b_local, parity) tile span within the call, core union
    tile_lo = np.zeros((NSG, NCHUNK, SGB, 2), np.int64)
    tile_hi = np.zeros((NSG, NCHUNK, SGB, 2), np.int64)
    for s in range(NSG):
        blo = (s * SGB) * 2
        for c in range(NCHUNK):
            st = bstart[:, s, c, blo : blo + SGB * 2]          # [NCORES, SGB*2]
            en = bend[:, s, c, blo : blo + SGB * 2]
            ncols = int(ucall[s, c]) // P
            lo = np.minimum((st // P).min(axis=0), ncols - 1)
            hi = np.minimum(((en + P - 1) // P).max(axis=0), ncols)
            hi = np.maximum(hi, lo + 1)
            tile_lo[s, c] = lo.reshape(SGB, 2)
            tile_hi[s, c] = hi.reshape(SGB, 2)

    # int16 wrap: element i -> [i % 16, i // 16], replicated to 128 partitions
    idxw = slot_idx.reshape(NCORES, cap_total // 16, 16).transpose(0, 2, 1)
    idxw = np.tile(idxw, (1, 8, 1)).copy()                     # [NCORES, 128, cap/16]

    # dstw: element i -> [i % 128, i // 128]
    dstw = slot_dpos.reshape(NCORES, cols, P).transpose(0, 2, 1).astype(np.float16)

    return {
        "ucall": ucall,
        "call_base": call_base,
        "cols": cols,
        "tile_lo": tile_lo,
        "tile_hi": tile_hi,
        "idxw": idxw,
        "dstw": dstw,
    }


@with_exitstack
def _build(ctx: ExitStack, tc, plan, eps_scale: float):
    nc = tc.nc
    f32 = mybir.dt.float32
    bf16 = mybir.dt.float16

    ucall = plan["ucall"]
    call_base = plan["call_base"]
    cols = plan["cols"]
    tile_lo = plan["tile_lo"]
    tile_hi = plan["tile_hi"]

    tab = nc.dram_tensor("tab", [TAB_ROWS, 2 * D], bf16, kind="ExternalInput").ap()
    dstw_in = nc.dram_tensor("dstw", [P, cols], bf16, kind="ExternalInput").ap()
    x_in = nc.dram_tensor("x", [NPC, D], f32, kind="ExternalInput").ap()
    idxw_in = nc.dram_tensor("idxw", [P, cols * 8], mybir.dt.int16, kind="ExternalInput").ap()
    w1_in = nc.dram_tensor("w1", [D, DH], bf16, kind="ExternalInput").ap()
    b1_in = nc.dram_tensor("b1", [DH, 1], f32, kind="ExternalInput").ap()
    w2_in = nc.dram_tensor("w2", [DH, D], bf16, kind="ExternalInput").ap()
    b2_in = nc.dram_tensor("b2", [D, 1], f32, kind="ExternalInput").ap()
    ident_in = nc.dram_tensor("ident", [P, P], bf16, kind="ExternalInput").ap()
    iota_in = nc.dram_tensor("iota", [P, 2 * SGB * P], bf16, kind="ExternalInput").ap()
    out = nc.dram_tensor("out", [NPC, D], f32, kind="ExternalOutput").ap()

    consts = ctx.enter_context(tc.tile_pool(name="consts", bufs=1))
    xpool = ctx.enter_context(tc.tile_pool(name="xp", bufs=2))
    hxpool = ctx.enter_context(tc.tile_pool(name="hxp", bufs=2))
    idxpool = ctx.enter_context(tc.tile_pool(name="idxp", bufs=6))
    dwpool = ctx.enter_context(tc.tile_pool(name="dwp", bufs=6))
    mpool = ctx.enter_context(tc.tile_pool(name="msgs", bufs=8))
    ohpool = ctx.enter_context(tc.tile_pool(name="oh", bufs=8))
    hpool = ctx.enter_context(tc.tile_pool(name="hp", bufs=3))
    sbp = ctx.enter_context(tc.tile_pool(name="sbp", bufs=3))
    psagg = ctx.enter_context(tc.tile_pool(name="psagg", bufs=2, space="PSUM"))
    pstp = ctx.enter_context(tc.tile_pool(name="pstp", bufs=1, space="PSUM"))
    psz1 = ctx.enter_context(tc.tile_pool(name="psz1", bufs=2, space="PSUM"))
    psoT = ctx.enter_context(tc.tile_pool(name="psoT", bufs=2, space="PSUM"))

    w1s = consts.tile([D, DH], bf16)
    nc.sync.dma_start(w1s[:], w1_in[:])
    w2s = consts.tile([DH, D], bf16)
    nc.sync.dma_start(w2s[:], w2_in[:])
    b1s = consts.tile([DH, 1], f32)
    nc.sync.dma_start(b1s[:], b1_in[:])
    b2s = consts.tile([D, 1], f32)
    nc.sync.dma_start(b2s[:], b2_in[:])
    ident = consts.tile([P, P], bf16)
    nc.sync.dma_start(ident[:], ident_in[:])
    # iota[:, i] = i; bucket (b, par) compares against [896*par + 128*b, +128)
    iota = consts.tile([P, 2 * SGB * P], bf16)
    nc.sync.dma_start(iota[:], iota_in[:])

    max_ccols = int(ucall.max()) // P
    qn = 0
    for s in range(NSG):
        # x slab for this supergroup, scaled by (1+eps)
        xs = xpool.tile([P, SGB * D], f32, tag="xs")
        nc.sync.dma_start(
            xs[:].rearrange("p (b f) -> p b f", b=SGB, f=D),
            x_in[s * SGB * P : (s + 1) * SGB * P, :].rearrange("(b p) f -> p b f", p=P),
        )
        if eps_scale == 1.0:
            hx = xs
        else:
            hx = hxpool.tile([P, SGB * D], f32, tag="hx")
            nc.vector.tensor_scalar_mul(hx[:], xs[:], eps_scale)

        msgs_c = []
        oh_c = []
        seg_off_c = []
        for c in range(NCHUNK):
            n_call = int(ucall[s, c])
            ccols = n_call // P
            idx_t = idxpool.tile([P, max(1, n_call // 16)], mybir.dt.int16, tag="idx")
            c16 = int(call_base[s, c]) // 16
            nc.sync.dma_start(idx_t[:], idxw_in[:, c16 : c16 + n_call // 16])
            dw_t = dwpool.tile([P, max_ccols], bf16, tag="dw")
            c128 = int(call_base[s, c]) // P
            nc.sync.dma_start(dw_t[:, :ccols], dstw_in[:, c128 : c128 + ccols])

            msgs = mpool.tile([P, max_ccols * 2 * D], bf16, tag="m")
            nc.gpsimd.dma_gather(
                out_ap=msgs[:, : ccols * 2 * D].rearrange(
                    "p (t f) -> p t f", t=ccols, f=2 * D
                ),
                in_ap=tab[c * (NPAIR + 1) : (c + 1) * (NPAIR + 1), :],
                idxs_ap=idx_t[:, : n_call // 16],
                num_idxs=n_call,
                num_idxs_reg=n_call,
                elem_size=2 * D,
                single_packet=False,
                queue_num=qn,
            )
            qn = (qn + 1) % NQ

            # one-hot segments, one per (window, parity) bucket
            spans = [
                [int(tile_hi[s, c, b, p] - tile_lo[s, c, b, p]) for p in range(2)]
                for b in range(SGB)
            ]
            seg_off = [[0, 0] for _ in range(SGB)]
            off = 0
            for b in range(SGB):
                for p in range(2):
                    seg_off[b][p] = off
                    off += spans[b][p] * P
            oh = ohpool.tile([P, max(P, off)], mybir.dt.float8e4, tag="oh")
            for b in range(SGB):
                for p in range(2):
                    t_lo = int(tile_lo[s, c, b, p])
                    span = spans[b][p]
                    if span <= 0:
                        continue
                    seg = oh[:, seg_off[b][p] : seg_off[b][p] + span * P]
                    nc.vector.tensor_tensor(
                        out=seg.rearrange("p (t j) -> p t j", t=span, j=P),
                        in0=dw_t[:, t_lo : t_lo + span]
                        .rearrange("p (t o) -> p t o", o=1)
                        .to_broadcast([P, span, P]),
                        in1=iota[:, (128 * SGB) * p + b * P : (128 * SGB) * p + (b + 1) * P]
                        .rearrange("p (o j) -> p o j", o=1)
                        .to_broadcast([P, span, P]),
                        op=mybir.AluOpType.is_equal,
                    )
            msgs_c.append(msgs)
            oh_c.append(oh)
            seg_off_c.append(seg_off)

        for b in range(SGB):
            blk = s * SGB + b
            total_mm = sum(
                int(tile_hi[s, c, b, p] - tile_lo[s, c, b, p])
                for c in range(NCHUNK)
                for p in range(2)
            )
            agg = psagg.tile([P, D], f32, tag="agg")
            done = 0
            for c in range(NCHUNK):
                for p in range(2):
                    t_lo = int(tile_lo[s, c, b, p])
                    span = int(tile_hi[s, c, b, p] - t_lo)
                    so = seg_off_c[c][b][p]
                    for t in range(span):
                        col = t_lo + t
                        nc.tensor.matmul(
                            out=agg[:],
                            lhsT=oh_c[c][:, so + t * P : so + (t + 1) * P],
                            rhs=msgs_c[c][
                                :, col * 2 * D + p * D : col * 2 * D + (p + 1) * D
                            ],
                            start=(done == 0),
                            stop=(done == total_mm - 1),
                        )
                        done += 1

            h = hpool.tile([P, D], bf16, tag="h")
            nc.vector.tensor_add(h[:], agg[:], hx[:, b * D : (b + 1) * D])

            hT_ps = pstp.tile([D, P], bf16, tag="tp")
            nc.tensor.transpose(out=hT_ps[:], in_=h[:], identity=ident[:])
            hT = sbp.tile([D, P], bf16, tag="hT")
            nc.scalar.copy(hT[:], hT_ps[:])

            z1_ps = psz1.tile([DH, P], f32, tag="z1")
            nc.tensor.matmul(out=z1_ps[:], lhsT=w1s[:], rhs=hT[:], start=True, stop=True)
            z1r = sbp.tile([DH, P], bf16, tag="z1r")
            nc.scalar.activation(z1r[:], z1_ps[:], mybir.ActivationFunctionType.Relu, bias=b1s[:])

            oT_ps = psoT.tile([D, P], f32, tag="oT")
            nc.tensor.matmul(out=oT_ps[:], lhsT=w2s[:], rhs=z1r[:], start=True, stop=True)
            oT = sbp.tile([D, P], bf16, tag="oTs")
            nc.scalar.activation(oT[:], oT_ps[:], mybir.ActivationFunctionType.Identity, bias=b2s[:])

            ob_ps = pstp.tile([P, D], bf16, tag="tp2")
            nc.tensor.transpose(out=ob_ps[:], in_=oT[:], identity=ident[:D, :D])
            ob = sbp.tile([P, D], f32, tag="ob")
            nc.scalar.copy(ob[:], ob_ps[:])
            nc.sync.dma_start(out[blk * P : (blk + 1) * P, :], ob[:])


def kernel(node_attr, W1, b1, W2, b2, eps, edge_src, edge_dst):
    import ml_dtypes

    bf = np.float16
    node_attr = np.asarray(node_attr, np.float32)
    W1 = np.asarray(W1, np.float32)
    b1 = np.asarray(b1, np.float32)
    W2 = np.asarray(W2, np.float32)
    b2 = np.asarray(b2, np.float32)
    eps_scale = 1.0 + float(np.asarray(eps))
    edge_src = np.asarray(edge_src, np.int32)
    edge_dst = np.asarray(edge_dst, np.int32)

    plan = _plan(edge_src, edge_dst)

    # chunked bf16 pair table with a zeros row per chunk
    tab = np.zeros((TAB_ROWS, 2 * D), bf)
    xb = node_attr.astype(bf)
    for c in range(NCHUNK):
        tab[c * (NPAIR + 1) : c * (NPAIR + 1) + NPAIR] = xb[
            c * CH : (c + 1) * CH
        ].reshape(NPAIR, 2 * D)

    x_pad = np.zeros((NCORES * NPC, D), np.float32)
    x_pad[:N] = node_attr

    ident = np.eye(P, dtype=bf)
    iota = np.tile(np.arange(2 * SGB * P, dtype=np.float16), (P, 1)).copy()

    import concourse.bacc as bacc

    nc = bacc.Bacc(
        "TRN2",
        target_bir_lowering=False,
        debug=False,
        num_devices=NCORES,
        num_swdge_queues=NQ,
    )
    with tile.TileContext(nc) as t:
        _build(t, plan, eps_scale)
    nc.compile()

    in_maps = []
    for c in range(NCORES):
        in_maps.append(
            {
                "tab": tab,
                "x": x_pad[c * NPC : (c + 1) * NPC],
                "idxw": plan["idxw"][c],
                "dstw": plan["dstw"][c],
                "w1": W1.astype(bf),
                "b1": b1.reshape(DH, 1),
                "w2": W2.astype(bf),
                "b2": b2.reshape(D, 1),
                "ident": ident,
                "iota": iota,
            }
        )

    res = bass_utils.run_bass_kernel_spmd(nc, in_maps, core_ids=list(range(NCORES)))
    import os as _os

    if res.exec_time_ns is not None:
        _os.environ["KERNEL_EXEC_NS"] = str(res.exec_time_ns)
    if res.instructions_and_trace is not None:
        print("trace:", res.instructions_and_trace[1])
    full = np.concatenate([r["out"] for r in res.results], axis=0)
    return full[:N]


if __name__ == "__main__":
    rng = np.random.default_rng(0)
    na = rng.normal(size=(N, D)).astype(np.float32)
    W1 = rng.normal(size=(D, DH)).astype(np.float32)
    b1 = np.zeros(DH, np.float32)
    W2 = rng.normal(size=(DH, D)).astype(np.float32)
    b2 = np.zeros(D, np.float32)
    eps = np.zeros((), np.float32)
    es = rng.integers(0, N, size=E).astype(np.int32)
    ed = rng.integers(0, N, size=E).astype(np.int32)
    out = kernel(na, W1, b1, W2, b2, eps, es, ed)
    print(out.shape, out.dtype)


# revision 32
# speedup vs baseline: 1.3628x; 1.3628x over previous
"""GSNConv (GIN message passing) Bass kernel for Trainium2, 8 NeuronCores.

Strategy (dst-sharded, gather-based, v3):
  - Nodes sharded by destination across 8 cores (12544 per core, 98 blocks
    of 128 = "windows").
  - node_attr stored as a bf16 PAIR table: row k = concat(x[2k], x[2k+1])
    (256 B rows satisfy dma_gather's elem-size constraint at bf16). Each
    edge gathers its src pair; the correct half is selected by a static
    column slice chosen via the edge's src parity.
  - Per core, edges sorted by (supergroup of 7 windows, src chunk, window,
    parity). One dma_gather per (supergroup, chunk), call lengths unified
    across cores, padded to 128 with zeros-row gathers; calls round-robin
    4 SWDGE queues (descriptor generation on gpsimd is the critical path).
  - No per-bucket padding: tiles may straddle (window, parity) boundaries;
    straddling tiles get matmuls for each bucket they touch, with one-hots
    built from block-shifted iota vs in-supergroup dst position (is_equal).
  - Aggregation matmuls and the 2-layer MLP run in bf16 (fp32 PSUM
    accumulation); fp32 is 4x slower on the PE.
"""

from contextlib import ExitStack

import numpy as np

import concourse.bass as bass
import concourse.tile as tile
from concourse import bass_utils, mybir
from concourse._compat import with_exitstack

# Problem shape (hardcoded per contract).
N = 100000
E = 1600000
D = 64
DH = 128
P = 128

NCORES = 8
NPC = 12544               # nodes per core
NWIN = NPC // P           # 98 windows (= output blocks) per core
SGB = 7                   # windows (=blocks) per supergroup
NSG = NWIN // SGB         # 14 supergroups
CH = 25000                # src chunk rows
NCHUNK = 4
TAB_ROWS = NCHUNK * (CH + 1)  # padded fp16 rows: [x[n] | zeros]
NQ = 4                    # SWDGE queues


def _plan(edge_src: np.ndarray, edge_dst: np.ndarray):
    """Sort/bucket edges; build per-core int16-wrapped pair-idx + dstw arrays,
    unified call lengths and per-(call, window, parity) tile spans."""
    src = edge_src.astype(np.int64)
    dst = edge_dst.astype(np.int64)

    core = dst // NPC
    ldst = dst - core * NPC
    w = ldst // P
    sg = w // SGB
    b_local = w - sg * SGB
    chunk = src // CH
    sloc = src - chunk * CH
    # in-supergroup dst position
    dpos = 128 * b_local + (ldst - w * P)

    # sort edges by (core, sg, chunk, w)
    key = (((core * NSG + sg) * NCHUNK + chunk) * NWIN + w).astype(np.int64)
    order = np.argsort(key, kind="stable")
    key_s = key[order]
    pidx_s = sloc[order]
    dpos_s = dpos[order]

    nb = NWIN                                                 # window buckets
    ngroups = NCORES * NSG * NCHUNK * nb
    counts = np.bincount(key_s, minlength=ngroups).reshape(NCORES, NSG, NCHUNK, nb)
    call_core = counts.sum(axis=3)                            # [NCORES, NSG, NCHUNK]
    ucall = call_core.max(axis=0)
    ucall = ((ucall + P - 1) // P) * P
    ucall = np.maximum(ucall, P)

    call_base = np.zeros((NSG, NCHUNK), np.int64)
    pos = 0
    for s in range(NSG):
        for c in range(NCHUNK):
            call_base[s, c] = pos
            pos += ucall[s, c]
    cap_total = int(pos)
    cols = cap_total // P

    # per-core bucket starts within the call
    bstart = np.zeros((NCORES, NSG, NCHUNK, nb), np.int64)
    np.cumsum(counts[:, :, :, :-1], axis=3, out=bstart[:, :, :, 1:])
    bend = bstart + counts

    # per-edge slot position
    starts = np.zeros(ngroups + 1, np.int64)
    np.cumsum(np.bincount(key_s, minlength=ngroups), out=starts[1:])
    rank = np.arange(len(key_s)) - starts[key_s]
    core_s = key_s // (NSG * NCHUNK * nb)
    rem = key_s - core_s * (NSG * NCHUNK * nb)
    sg_s = rem // (NCHUNK * nb)
    rem2 = rem - sg_s * (NCHUNK * nb)
    c_s = rem2 // nb
    b_s = rem2 - c_s * nb
    pos_edge = call_base[sg_s, c_s] + bstart[core_s, sg_s, c_s, b_s] + rank

    slot_idx = np.full((NCORES, cap_total), CH, np.int16)       # pad -> zeros row
    slot_dpos = np.full((NCORES, cap_total), -1.0, np.float32)  # pad -> no window
    slot_idx[core_s, pos_edge] = pidx_s.astype(np.int16)
    slot_dpos[core_s, pos_edge] = dpos_s.astype(np.float32)

    # per-(sg, chunk, b_local) tile span within the call, core union
    tile_lo = np.zeros((NSG, NCHUNK, SGB), np.int64)
    tile_hi = np.zeros((NSG, NCHUNK, SGB), np.int64)
    for s in range(NSG):
        blo = s * SGB
        for c in range(NCHUNK):
            st = bstart[:, s, c, blo : blo + SGB]              # [NCORES, SGB]
            en = bend[:, s, c, blo : blo + SGB]
            ncols = int(ucall[s, c]) // P
            lo = np.minimum((st // P).min(axis=0), ncols - 1)
            hi = np.minimum(((en + P - 1) // P).max(axis=0), ncols)
            hi = np.maximum(hi, lo + 1)
            tile_lo[s, c] = lo
            tile_hi[s, c] = hi

    # int16 wrap: element i -> [i % 16, i // 16], replicated to 128 partitions
    idxw = slot_idx.reshape(NCORES, cap_total // 16, 16).transpose(0, 2, 1)
    idxw = np.tile(idxw, (1, 8, 1)).copy()                     # [NCORES, 128, cap/16]

    # dstw: element i -> [i % 128, i // 128]
    dstw = slot_dpos.reshape(NCORES, cols, P).transpose(0, 2, 1).astype(np.float16)

    return {
        "ucall": ucall,
        "call_base": call_base,
        "cols": cols,
        "tile_lo": tile_lo,
        "tile_hi": tile_hi,
        "idxw": idxw,
        "dstw": dstw,
    }


@with_exitstack
def _build(ctx: ExitStack, tc, plan, eps_scale: float):
    nc = tc.nc
    f32 = mybir.dt.float32
    bf16 = mybir.dt.float16

    ucall = plan["ucall"]
    call_base = plan["call_base"]
    cols = plan["cols"]
    tile_lo = plan["tile_lo"]
    tile_hi = plan["tile_hi"]

    tab = nc.dram_tensor("tab", [TAB_ROWS, 2 * D], bf16, kind="ExternalInput").ap()
    dstw_in = nc.dram_tensor("dstw", [P, cols], bf16, kind="ExternalInput").ap()
    x_in = nc.dram_tensor("x", [NPC, D], f32, kind="ExternalInput").ap()
    idxw_in = nc.dram_tensor("idxw", [P, cols * 8], mybir.dt.int16, kind="ExternalInput").ap()
    w1_in = nc.dram_tensor("w1", [D, DH], bf16, kind="ExternalInput").ap()
    b1_in = nc.dram_tensor("b1", [DH, 1], f32, kind="ExternalInput").ap()
    w2_in = nc.dram_tensor("w2", [DH, D], bf16, kind="ExternalInput").ap()
    b2_in = nc.dram_tensor("b2", [D, 1], f32, kind="ExternalInput").ap()
    ident_in = nc.dram_tensor("ident", [P, P], bf16, kind="ExternalInput").ap()
    iota_in = nc.dram_tensor("iota", [P, SGB * P], bf16, kind="ExternalInput").ap()
    out = nc.dram_tensor("out", [NPC, D], f32, kind="ExternalOutput").ap()

    consts = ctx.enter_context(tc.tile_pool(name="consts", bufs=1))
    xpool = ctx.enter_context(tc.tile_pool(name="xp", bufs=2))
    hxpool = ctx.enter_context(tc.tile_pool(name="hxp", bufs=2))
    idxpool = ctx.enter_context(tc.tile_pool(name="idxp", bufs=6))
    dwpool = ctx.enter_context(tc.tile_pool(name="dwp", bufs=6))
    mpool = ctx.enter_context(tc.tile_pool(name="msgs", bufs=8))
    ohpool = ctx.enter_context(tc.tile_pool(name="oh", bufs=8))
    hpool = ctx.enter_context(tc.tile_pool(name="hp", bufs=3))
    sbp = ctx.enter_context(tc.tile_pool(name="sbp", bufs=3))
    psagg = ctx.enter_context(tc.tile_pool(name="psagg", bufs=2, space="PSUM"))
    pstp = ctx.enter_context(tc.tile_pool(name="pstp", bufs=1, space="PSUM"))
    psz1 = ctx.enter_context(tc.tile_pool(name="psz1", bufs=2, space="PSUM"))
    psoT = ctx.enter_context(tc.tile_pool(name="psoT", bufs=2, space="PSUM"))

    w1s = consts.tile([D, DH], bf16)
    nc.sync.dma_start(w1s[:], w1_in[:])
    w2s = consts.tile([DH, D], bf16)
    nc.sync.dma_start(w2s[:], w2_in[:])
    b1s = consts.tile([DH, 1], f32)
    nc.sync.dma_start(b1s[:], b1_in[:])
    b2s = consts.tile([D, 1], f32)
    nc.sync.dma_start(b2s[:], b2_in[:])
    ident = consts.tile([P, P], bf16)
    nc.sync.dma_start(ident[:], ident_in[:])
    # iota[:, i] = i; window b compares against [128*b, 128*b + 128)
    iota = consts.tile([P, SGB * P], bf16)
    nc.sync.dma_start(iota[:], iota_in[:])

    max_ccols = int(ucall.max()) // P
    qn = 0
    for s in range(NSG):
        # x slab for this supergroup, scaled by (1+eps)
        xs = xpool.tile([P, SGB * D], f32, tag="xs")
        nc.sync.dma_start(
            xs[:].rearrange("p (b f) -> p b f", b=SGB, f=D),
            x_in[s * SGB * P : (s + 1) * SGB * P, :].rearrange("(b p) f -> p b f", p=P),
        )
        if eps_scale == 1.0:
            hx = xs
        else:
            hx = hxpool.tile([P, SGB * D], f32, tag="hx")
            nc.vector.tensor_scalar_mul(hx[:], xs[:], eps_scale)

        msgs_c = []
        oh_c = []
        seg_off_c = []
        for c in range(NCHUNK):
            n_call = int(ucall[s, c])
            ccols = n_call // P
            idx_t = idxpool.tile([P, max(1, n_call // 16)], mybir.dt.int16, tag="idx")
            c16 = int(call_base[s, c]) // 16
            nc.sync.dma_start(idx_t[:], idxw_in[:, c16 : c16 + n_call // 16])
            dw_t = dwpool.tile([P, max_ccols], bf16, tag="dw")
            c128 = int(call_base[s, c]) // P
            nc.sync.dma_start(dw_t[:, :ccols], dstw_in[:, c128 : c128 + ccols])

            msgs = mpool.tile([P, max_ccols * 2 * D], bf16, tag="m")
            nc.gpsimd.dma_gather(
                out_ap=msgs[:, : ccols * 2 * D].rearrange(
                    "p (t f) -> p t f", t=ccols, f=2 * D
                ),
                in_ap=tab[c * (CH + 1) : (c + 1) * (CH + 1), :],
                idxs_ap=idx_t[:, : n_call // 16],
                num_idxs=n_call,
                num_idxs_reg=n_call,
                elem_size=2 * D,
                single_packet=False,
                queue_num=qn,
            )
            qn = (qn + 1) % NQ

            # one-hot segments, one per window bucket
            spans = [int(tile_hi[s, c, b] - tile_lo[s, c, b]) for b in range(SGB)]
            seg_off = []
            off = 0
            for b in range(SGB):
                seg_off.append(off)
                off += spans[b] * P
            oh = ohpool.tile([P, max(P, off)], mybir.dt.float8e4, tag="oh")
            for b in range(SGB):
                t_lo = int(tile_lo[s, c, b])
                span = spans[b]
                if span <= 0:
                    continue
                seg = oh[:, seg_off[b] : seg_off[b] + span * P]
                nc.vector.tensor_tensor(
                    out=seg.rearrange("p (t j) -> p t j", t=span, j=P),
                    in0=dw_t[:, t_lo : t_lo + span]
                    .rearrange("p (t o) -> p t o", o=1)
                    .to_broadcast([P, span, P]),
                    in1=iota[:, b * P : (b + 1) * P]
                    .rearrange("p (o j) -> p o j", o=1)
                    .to_broadcast([P, span, P]),
                    op=mybir.AluOpType.is_equal,
                )
            msgs_c.append(msgs)
            oh_c.append(oh)
            seg_off_c.append(seg_off)

        for b in range(SGB):
            blk = s * SGB + b
            total_mm = sum(
                int(tile_hi[s, c, b] - tile_lo[s, c, b]) for c in range(NCHUNK)
            )
            agg = psagg.tile([P, D], f32, tag="agg")
            done = 0
            for c in range(NCHUNK):
                t_lo = int(tile_lo[s, c, b])
                span = int(tile_hi[s, c, b] - t_lo)
                so = seg_off_c[c][b]
                for t in range(span):
                    col = t_lo + t
                    nc.tensor.matmul(
                        out=agg[:],
                        lhsT=oh_c[c][:, so + t * P : so + (t + 1) * P],
                        rhs=msgs_c[c][:, col * 2 * D : col * 2 * D + D],
                        start=(done == 0),
                        stop=(done == total_mm - 1),
                    )
                    done += 1

            h = hpool.tile([P, D], bf16, tag="h")
            nc.vector.tensor_add(h[:], agg[:], hx[:, b * D : (b + 1) * D])

            hT_ps = pstp.tile([D, P], bf16, tag="tp")
            nc.tensor.transpose(out=hT_ps[:], in_=h[:], identity=ident[:])
            hT = sbp.tile([D, P], bf16, tag="hT")
            nc.scalar.copy(hT[:], hT_ps[:])

            z1_ps = psz1.tile([DH, P], f32, tag="z1")
            nc.tensor.matmul(out=z1_ps[:], lhsT=w1s[:], rhs=hT[:], start=True, stop=True)
            z1r = sbp.tile([DH, P], bf16, tag="z1r")
            nc.scalar.activation(z1r[:], z1_ps[:], mybir.ActivationFunctionType.Relu, bias=b1s[:])

            oT_ps = psoT.tile([D, P], f32, tag="oT")
            nc.tensor.matmul(out=oT_ps[:], lhsT=w2s[:], rhs=z1r[:], start=True, stop=True)
            oT = sbp.tile([D, P], bf16, tag="oTs")
            nc.scalar.activation(oT[:], oT_ps[:], mybir.ActivationFunctionType.Identity, bias=b2s[:])

            ob_ps = pstp.tile([P, D], bf16, tag="tp2")
            nc.tensor.transpose(out=ob_ps[:], in_=oT[:], identity=ident[:D, :D])
            ob = sbp.tile([P, D], f32, tag="ob")
            nc.scalar.copy(ob[:], ob_ps[:])
            nc.sync.dma_start(out[blk * P : (blk + 1) * P, :], ob[:])


def kernel(node_attr, W1, b1, W2, b2, eps, edge_src, edge_dst):
    import ml_dtypes

    bf = np.float16
    node_attr = np.asarray(node_attr, np.float32)
    W1 = np.asarray(W1, np.float32)
    b1 = np.asarray(b1, np.float32)
    W2 = np.asarray(W2, np.float32)
    b2 = np.asarray(b2, np.float32)
    eps_scale = 1.0 + float(np.asarray(eps))
    edge_src = np.asarray(edge_src, np.int32)
    edge_dst = np.asarray(edge_dst, np.int32)

    plan = _plan(edge_src, edge_dst)

    # chunked fp16 table, rows padded to 256 B: [x[n] | zeros], zeros row per chunk
    tab = np.zeros((TAB_ROWS, 2 * D), bf)
    xb = node_attr.astype(bf)
    for c in range(NCHUNK):
        tab[c * (CH + 1) : c * (CH + 1) + CH, :D] = xb[c * CH : (c + 1) * CH]

    x_pad = np.zeros((NCORES * NPC, D), np.float32)
    x_pad[:N] = node_attr

    ident = np.eye(P, dtype=bf)
    iota = np.tile(np.arange(SGB * P, dtype=np.float16), (P, 1)).copy()

    import concourse.bacc as bacc

    nc = bacc.Bacc(
        "TRN2",
        target_bir_lowering=False,
        debug=False,
        num_devices=NCORES,
        num_swdge_queues=NQ,
    )
    with tile.TileContext(nc) as t:
        _build(t, plan, eps_scale)
    nc.compile()

    in_maps = []
    for c in range(NCORES):
        in_maps.append(
            {
                "tab": tab,
                "x": x_pad[c * NPC : (c + 1) * NPC],
                "idxw": plan["idxw"][c],
                "dstw": plan["dstw"][c],
                "w1": W1.astype(bf),
                "b1": b1.reshape(DH, 1),
                "w2": W2.astype(bf),
                "b2": b2.reshape(D, 1),
                "ident": ident,
                "iota": iota,
            }
        )

    res = bass_utils.run_bass_kernel_spmd(nc, in_maps, core_ids=list(range(NCORES)))
    import os as _os

    if res.exec_time_ns is not None:
        _os.environ["KERNEL_EXEC_NS"] = str(res.exec_time_ns)
    if res.instructions_and_trace is not None:
        print("trace:", res.instructions_and_trace[1])
    full = np.concatenate([r["out"] for r in res.results], axis=0)
    return full[:N]


if __name__ == "__main__":
    rng = np.random.default_rng(0)
    na = rng.normal(size=(N, D)).astype(np.float32)
    W1 = rng.normal(size=(D, DH)).astype(np.float32)
    b1 = np.zeros(DH, np.float32)
    W2 = rng.normal(size=(DH, D)).astype(np.float32)
    b2 = np.zeros(D, np.float32)
    eps = np.zeros((), np.float32)
    es = rng.integers(0, N, size=E).astype(np.int32)
    ed = rng.integers(0, N, size=E).astype(np.int32)
    out = kernel(na, W1, b1, W2, b2, eps, es, ed)
    print(out.shape, out.dtype)


# revision 33
# speedup vs baseline: 1.3996x; 1.0270x over previous
"""GSNConv (GIN message passing) Bass kernel for Trainium2, 8 NeuronCores.

Strategy (dst-sharded, gather-based, v3):
  - Nodes sharded by destination across 8 cores (12544 per core, 98 blocks
    of 128 = "windows").
  - node_attr stored as a bf16 PAIR table: row k = concat(x[2k], x[2k+1])
    (256 B rows satisfy dma_gather's elem-size constraint at bf16). Each
    edge gathers its src pair; the correct half is selected by a static
    column slice chosen via the edge's src parity.
  - Per core, edges sorted by (supergroup of 7 windows, src chunk, window,
    parity). One dma_gather per (supergroup, chunk), call lengths unified
    across cores, padded to 128 with zeros-row gathers; calls round-robin
    4 SWDGE queues (descriptor generation on gpsimd is the critical path).
  - No per-bucket padding: tiles may straddle (window, parity) boundaries;
    straddling tiles get matmuls for each bucket they touch, with one-hots
    built from block-shifted iota vs in-supergroup dst position (is_equal).
  - Aggregation matmuls and the 2-layer MLP run in bf16 (fp32 PSUM
    accumulation); fp32 is 4x slower on the PE.
"""

from contextlib import ExitStack

import numpy as np

import concourse.bass as bass
import concourse.tile as tile
from concourse import bass_utils, mybir
from concourse._compat import with_exitstack

# Problem shape (hardcoded per contract).
N = 100000
E = 1600000
D = 64
DH = 128
P = 128

NCORES = 8
NPC = 12544               # nodes per core
NWIN = NPC // P           # 98 windows (= output blocks) per core
SGB = 7                   # windows (=blocks) per supergroup
NSG = NWIN // SGB         # 14 supergroups
CH = 25000                # src chunk rows
NCHUNK = 4
TAB_ROWS = NCHUNK * (CH + 1)  # padded fp16 rows: [x[n] | zeros]
NQ = 4                    # SWDGE queues


def _plan(edge_src: np.ndarray, edge_dst: np.ndarray):
    """Sort/bucket edges; build per-core int16-wrapped pair-idx + dstw arrays,
    unified call lengths and per-(call, window, parity) tile spans."""
    src = edge_src.astype(np.int64)
    dst = edge_dst.astype(np.int64)

    core = dst // NPC
    ldst = dst - core * NPC
    w = ldst // P
    sg = w // SGB
    b_local = w - sg * SGB
    chunk = src // CH
    sloc = src - chunk * CH
    # in-supergroup dst position
    dpos = 128 * b_local + (ldst - w * P)

    # sort edges by (core, sg, chunk, w)
    key = (((core * NSG + sg) * NCHUNK + chunk) * NWIN + w).astype(np.int64)
    order = np.argsort(key, kind="stable")
    key_s = key[order]
    pidx_s = sloc[order]
    dpos_s = dpos[order]

    nb = NWIN                                                 # window buckets
    ngroups = NCORES * NSG * NCHUNK * nb
    counts = np.bincount(key_s, minlength=ngroups).reshape(NCORES, NSG, NCHUNK, nb)
    call_core = counts.sum(axis=3)                            # [NCORES, NSG, NCHUNK]
    ucall = call_core.max(axis=0)
    ucall = ((ucall + P - 1) // P) * P
    ucall = np.maximum(ucall, P)

    call_base = np.zeros((NSG, NCHUNK), np.int64)
    pos = 0
    for s in range(NSG):
        for c in range(NCHUNK):
            call_base[s, c] = pos
            pos += ucall[s, c]
    cap_total = int(pos)
    cols = cap_total // P

    # per-core bucket starts within the call
    bstart = np.zeros((NCORES, NSG, NCHUNK, nb), np.int64)
    np.cumsum(counts[:, :, :, :-1], axis=3, out=bstart[:, :, :, 1:])
    bend = bstart + counts

    # per-edge slot position
    starts = np.zeros(ngroups + 1, np.int64)
    np.cumsum(np.bincount(key_s, minlength=ngroups), out=starts[1:])
    rank = np.arange(len(key_s)) - starts[key_s]
    core_s = key_s // (NSG * NCHUNK * nb)
    rem = key_s - core_s * (NSG * NCHUNK * nb)
    sg_s = rem // (NCHUNK * nb)
    rem2 = rem - sg_s * (NCHUNK * nb)
    c_s = rem2 // nb
    b_s = rem2 - c_s * nb
    pos_edge = call_base[sg_s, c_s] + bstart[core_s, sg_s, c_s, b_s] + rank

    slot_idx = np.full((NCORES, cap_total), CH, np.int16)       # pad -> zeros row
    slot_dpos = np.full((NCORES, cap_total), -1.0, np.float32)  # pad -> no window
    slot_idx[core_s, pos_edge] = pidx_s.astype(np.int16)
    slot_dpos[core_s, pos_edge] = dpos_s.astype(np.float32)

    # per-(sg, chunk, b_local) tile span within the call, core union
    tile_lo = np.zeros((NSG, NCHUNK, SGB), np.int64)
    tile_hi = np.zeros((NSG, NCHUNK, SGB), np.int64)
    for s in range(NSG):
        blo = s * SGB
        for c in range(NCHUNK):
            st = bstart[:, s, c, blo : blo + SGB]              # [NCORES, SGB]
            en = bend[:, s, c, blo : blo + SGB]
            ncols = int(ucall[s, c]) // P
            lo = np.minimum((st // P).min(axis=0), ncols - 1)
            hi = np.minimum(((en + P - 1) // P).max(axis=0), ncols)
            hi = np.maximum(hi, lo + 1)
            tile_lo[s, c] = lo
            tile_hi[s, c] = hi

    # int16 wrap: element i -> [i % 16, i // 16], replicated to 128 partitions
    idxw = slot_idx.reshape(NCORES, cap_total // 16, 16).transpose(0, 2, 1)
    idxw = np.tile(idxw, (1, 8, 1)).copy()                     # [NCORES, 128, cap/16]

    # dstw: element i -> [i % 128, i // 128]
    dstw = slot_dpos.reshape(NCORES, cols, P).transpose(0, 2, 1).astype(np.float16)

    return {
        "ucall": ucall,
        "call_base": call_base,
        "cols": cols,
        "tile_lo": tile_lo,
        "tile_hi": tile_hi,
        "idxw": idxw,
        "dstw": dstw,
    }


@with_exitstack
def _build(ctx: ExitStack, tc, plan, eps_scale: float):
    nc = tc.nc
    f32 = mybir.dt.float32
    bf16 = mybir.dt.float16

    ucall = plan["ucall"]
    call_base = plan["call_base"]
    cols = plan["cols"]
    tile_lo = plan["tile_lo"]
    tile_hi = plan["tile_hi"]

    tab = nc.dram_tensor("tab", [TAB_ROWS, 2 * D], bf16, kind="ExternalInput").ap()
    dstw_in = nc.dram_tensor("dstw", [P, cols], bf16, kind="ExternalInput").ap()
    x_in = nc.dram_tensor("x", [NSG * P, SGB * D], f32, kind="ExternalInput").ap()
    idxw_in = nc.dram_tensor("idxw", [P, cols * 8], mybir.dt.int16, kind="ExternalInput").ap()
    w1_in = nc.dram_tensor("w1", [D, DH], bf16, kind="ExternalInput").ap()
    b1_in = nc.dram_tensor("b1", [DH, 1], f32, kind="ExternalInput").ap()
    w2_in = nc.dram_tensor("w2", [DH, D], bf16, kind="ExternalInput").ap()
    b2_in = nc.dram_tensor("b2", [D, 1], f32, kind="ExternalInput").ap()
    ident_in = nc.dram_tensor("ident", [P, P], bf16, kind="ExternalInput").ap()
    iota_in = nc.dram_tensor("iota", [P, SGB * P], bf16, kind="ExternalInput").ap()
    out = nc.dram_tensor("out", [NSG * P, SGB * D], f32, kind="ExternalOutput").ap()

    consts = ctx.enter_context(tc.tile_pool(name="consts", bufs=1))
    xpool = ctx.enter_context(tc.tile_pool(name="xp", bufs=2))
    hxpool = ctx.enter_context(tc.tile_pool(name="hxp", bufs=2))
    obpool = ctx.enter_context(tc.tile_pool(name="obp", bufs=2))
    idxpool = ctx.enter_context(tc.tile_pool(name="idxp", bufs=6))
    dwpool = ctx.enter_context(tc.tile_pool(name="dwp", bufs=6))
    mpool = ctx.enter_context(tc.tile_pool(name="msgs", bufs=8))
    ohpool = ctx.enter_context(tc.tile_pool(name="oh", bufs=8))
    hpool = ctx.enter_context(tc.tile_pool(name="hp", bufs=3))
    sbp = ctx.enter_context(tc.tile_pool(name="sbp", bufs=3))
    psagg = ctx.enter_context(tc.tile_pool(name="psagg", bufs=2, space="PSUM"))
    pstp = ctx.enter_context(tc.tile_pool(name="pstp", bufs=1, space="PSUM"))
    psz1 = ctx.enter_context(tc.tile_pool(name="psz1", bufs=2, space="PSUM"))
    psoT = ctx.enter_context(tc.tile_pool(name="psoT", bufs=2, space="PSUM"))

    w1s = consts.tile([D, DH], bf16)
    nc.sync.dma_start(w1s[:], w1_in[:])
    w2s = consts.tile([DH, D], bf16)
    nc.sync.dma_start(w2s[:], w2_in[:])
    b1s = consts.tile([DH, 1], f32)
    nc.sync.dma_start(b1s[:], b1_in[:])
    b2s = consts.tile([D, 1], f32)
    nc.sync.dma_start(b2s[:], b2_in[:])
    ident = consts.tile([P, P], bf16)
    nc.sync.dma_start(ident[:], ident_in[:])
    # iota[:, i] = i; window b compares against [128*b, 128*b + 128)
    iota = consts.tile([P, SGB * P], bf16)
    nc.sync.dma_start(iota[:], iota_in[:])

    max_ccols = int(ucall.max()) // P
    qn = 0
    for s in range(NSG):
        # x slab for this supergroup, scaled by (1+eps)
        xs = xpool.tile([P, SGB * D], f32, tag="xs")
        nc.sync.dma_start(xs[:], x_in[s * P : (s + 1) * P, :])
        if eps_scale == 1.0:
            hx = xs
        else:
            hx = hxpool.tile([P, SGB * D], f32, tag="hx")
            nc.vector.tensor_scalar_mul(hx[:], xs[:], eps_scale)

        obsg = obpool.tile([P, SGB * D], f32, tag="obsg")
        msgs_c = []
        oh_c = []
        seg_off_c = []
        for c in range(NCHUNK):
            n_call = int(ucall[s, c])
            ccols = n_call // P
            idx_t = idxpool.tile([P, max(1, n_call // 16)], mybir.dt.int16, tag="idx")
            c16 = int(call_base[s, c]) // 16
            nc.sync.dma_start(idx_t[:], idxw_in[:, c16 : c16 + n_call // 16])
            dw_t = dwpool.tile([P, max_ccols], bf16, tag="dw")
            c128 = int(call_base[s, c]) // P
            nc.sync.dma_start(dw_t[:, :ccols], dstw_in[:, c128 : c128 + ccols])

            msgs = mpool.tile([P, max_ccols * 2 * D], bf16, tag="m")
            nc.gpsimd.dma_gather(
                out_ap=msgs[:, : ccols * 2 * D].rearrange(
                    "p (t f) -> p t f", t=ccols, f=2 * D
                ),
                in_ap=tab[c * (CH + 1) : (c + 1) * (CH + 1), :],
                idxs_ap=idx_t[:, : n_call // 16],
                num_idxs=n_call,
                num_idxs_reg=n_call,
                elem_size=2 * D,
                single_packet=False,
                queue_num=qn,
            )
            qn = (qn + 1) % NQ

            # one-hot segments, one per window bucket
            spans = [int(tile_hi[s, c, b] - tile_lo[s, c, b]) for b in range(SGB)]
            seg_off = []
            off = 0
            for b in range(SGB):
                seg_off.append(off)
                off += spans[b] * P
            oh = ohpool.tile([P, max(P, off)], mybir.dt.float8e4, tag="oh")
            for b in range(SGB):
                t_lo = int(tile_lo[s, c, b])
                span = spans[b]
                if span <= 0:
                    continue
                seg = oh[:, seg_off[b] : seg_off[b] + span * P]
                nc.vector.tensor_tensor(
                    out=seg.rearrange("p (t j) -> p t j", t=span, j=P),
                    in0=dw_t[:, t_lo : t_lo + span]
                    .rearrange("p (t o) -> p t o", o=1)
                    .to_broadcast([P, span, P]),
                    in1=iota[:, b * P : (b + 1) * P]
                    .rearrange("p (o j) -> p o j", o=1)
                    .to_broadcast([P, span, P]),
                    op=mybir.AluOpType.is_equal,
                )
            msgs_c.append(msgs)
            oh_c.append(oh)
            seg_off_c.append(seg_off)

        for b in range(SGB):
            blk = s * SGB + b
            total_mm = sum(
                int(tile_hi[s, c, b] - tile_lo[s, c, b]) for c in range(NCHUNK)
            )
            agg = psagg.tile([P, D], f32, tag="agg")
            done = 0
            for c in range(NCHUNK):
                t_lo = int(tile_lo[s, c, b])
                span = int(tile_hi[s, c, b] - t_lo)
                so = seg_off_c[c][b]
                for t in range(span):
                    col = t_lo + t
                    nc.tensor.matmul(
                        out=agg[:],
                        lhsT=oh_c[c][:, so + t * P : so + (t + 1) * P],
                        rhs=msgs_c[c][:, col * 2 * D : col * 2 * D + D],
                        start=(done == 0),
                        stop=(done == total_mm - 1),
                    )
                    done += 1

            h = hpool.tile([P, D], bf16, tag="h")
            nc.vector.tensor_add(h[:], agg[:], hx[:, b * D : (b + 1) * D])

            hT_ps = pstp.tile([D, P], bf16, tag="tp")
            nc.tensor.transpose(out=hT_ps[:], in_=h[:], identity=ident[:])
            hT = sbp.tile([D, P], bf16, tag="hT")
            nc.scalar.copy(hT[:], hT_ps[:])

            z1_ps = psz1.tile([DH, P], f32, tag="z1")
            nc.tensor.matmul(out=z1_ps[:], lhsT=w1s[:], rhs=hT[:], start=True, stop=True)
            z1r = sbp.tile([DH, P], bf16, tag="z1r")
            nc.scalar.activation(z1r[:], z1_ps[:], mybir.ActivationFunctionType.Relu, bias=b1s[:])

            oT_ps = psoT.tile([D, P], f32, tag="oT")
            nc.tensor.matmul(out=oT_ps[:], lhsT=w2s[:], rhs=z1r[:], start=True, stop=True)
            oT = sbp.tile([D, P], bf16, tag="oTs")
            nc.scalar.activation(oT[:], oT_ps[:], mybir.ActivationFunctionType.Identity, bias=b2s[:])

            ob_ps = pstp.tile([P, D], bf16, tag="tp2")
            nc.tensor.transpose(out=ob_ps[:], in_=oT[:], identity=ident[:D, :D])
            nc.scalar.copy(obsg[:, b * D : (b + 1) * D], ob_ps[:])
        nc.sync.dma_start(out[s * P : (s + 1) * P, :], obsg[:])


def kernel(node_attr, W1, b1, W2, b2, eps, edge_src, edge_dst):
    import ml_dtypes

    bf = np.float16
    node_attr = np.asarray(node_attr, np.float32)
    W1 = np.asarray(W1, np.float32)
    b1 = np.asarray(b1, np.float32)
    W2 = np.asarray(W2, np.float32)
    b2 = np.asarray(b2, np.float32)
    eps_scale = 1.0 + float(np.asarray(eps))
    edge_src = np.asarray(edge_src, np.int32)
    edge_dst = np.asarray(edge_dst, np.int32)

    plan = _plan(edge_src, edge_dst)

    # chunked fp16 table, rows padded to 256 B: [x[n] | zeros], zeros row per chunk
    tab = np.zeros((TAB_ROWS, 2 * D), bf)
    xb = node_attr.astype(bf)
    for c in range(NCHUNK):
        tab[c * (CH + 1) : c * (CH + 1) + CH, :D] = xb[c * CH : (c + 1) * CH]

    x_pad = np.zeros((NCORES * NPC, D), np.float32)
    x_pad[:N] = node_attr
    # per-core x, re-laid-out so each supergroup slab is a contiguous
    # [128, SGB*D] tile: row (s*128 + p) holds x[core_base + s*896 + b*128 + p]
    x_r = (
        x_pad.reshape(NCORES, NSG, SGB, P, D)
        .transpose(0, 1, 3, 2, 4)
        .reshape(NCORES, NSG * P, SGB * D)
    )

    ident = np.eye(P, dtype=bf)
    iota = np.tile(np.arange(SGB * P, dtype=np.float16), (P, 1)).copy()

    import concourse.bacc as bacc

    nc = bacc.Bacc(
        "TRN2",
        target_bir_lowering=False,
        debug=False,
        num_devices=NCORES,
        num_swdge_queues=NQ,
    )
    with tile.TileContext(nc) as t:
        _build(t, plan, eps_scale)
    nc.compile()

    in_maps = []
    for c in range(NCORES):
        in_maps.append(
            {
                "tab": tab,
                "x": x_r[c],
                "idxw": plan["idxw"][c],
                "dstw": plan["dstw"][c],
                "w1": W1.astype(bf),
                "b1": b1.reshape(DH, 1),
                "w2": W2.astype(bf),
                "b2": b2.reshape(D, 1),
                "ident": ident,
                "iota": iota,
            }
        )

    res = bass_utils.run_bass_kernel_spmd(nc, in_maps, core_ids=list(range(NCORES)))
    import os as _os

    if res.exec_time_ns is not None:
        _os.environ["KERNEL_EXEC_NS"] = str(res.exec_time_ns)
    if res.instructions_and_trace is not None:
        print("trace:", res.instructions_and_trace[1])
    outs = []
    for r in res.results:
        o = (
            r["out"]
            .reshape(NSG, P, SGB, D)
            .transpose(0, 2, 1, 3)
            .reshape(NPC, D)
        )
        outs.append(o)
    full = np.concatenate(outs, axis=0)
    return full[:N]


if __name__ == "__main__":
    rng = np.random.default_rng(0)
    na = rng.normal(size=(N, D)).astype(np.float32)
    W1 = rng.normal(size=(D, DH)).astype(np.float32)
    b1 = np.zeros(DH, np.float32)
    W2 = rng.normal(size=(DH, D)).astype(np.float32)
    b2 = np.zeros(D, np.float32)
    eps = np.zeros((), np.float32)
    es = rng.integers(0, N, size=E).astype(np.int32)
    ed = rng.integers(0, N, size=E).astype(np.int32)
    out = kernel(na, W1, b1, W2, b2, eps, es, ed)
    print(out.shape, out.dtype)


# revision 35
# speedup vs baseline: 1.4601x; 1.0433x over previous
"""GSNConv (GIN message passing) Bass kernel for Trainium2, 8 NeuronCores.

Strategy (dst-sharded, gather-based, v3):
  - Nodes sharded by destination across 8 cores (12544 per core, 98 blocks
    of 128 = "windows").
  - node_attr stored as a bf16 PAIR table: row k = concat(x[2k], x[2k+1])
    (256 B rows satisfy dma_gather's elem-size constraint at bf16). Each
    edge gathers its src pair; the correct half is selected by a static
    column slice chosen via the edge's src parity.
  - Per core, edges sorted by (supergroup of 7 windows, src chunk, window,
    parity). One dma_gather per (supergroup, chunk), call lengths unified
    across cores, padded to 128 with zeros-row gathers; calls round-robin
    4 SWDGE queues (descriptor generation on gpsimd is the critical path).
  - No per-bucket padding: tiles may straddle (window, parity) boundaries;
    straddling tiles get matmuls for each bucket they touch, with one-hots
    built from block-shifted iota vs in-supergroup dst position (is_equal).
  - Aggregation matmuls and the 2-layer MLP run in bf16 (fp32 PSUM
    accumulation); fp32 is 4x slower on the PE.
"""

from contextlib import ExitStack

import numpy as np

import concourse.bass as bass
import concourse.tile as tile
from concourse import bass_utils, mybir
from concourse._compat import with_exitstack

# Problem shape (hardcoded per contract).
N = 100000
E = 1600000
D = 64
DH = 128
P = 128

NCORES = 8
NPC = 12544               # nodes per core
NWIN = NPC // P           # 98 windows (= output blocks) per core
SGB = 7                   # windows (=blocks) per supergroup
NSG = NWIN // SGB         # 14 supergroups
CH = 25000                # src chunk rows
NCHUNK = 4
TAB_ROWS = NCHUNK * (CH + 1)  # padded fp16 rows: [x[n] | zeros]
NQ = 4                    # SWDGE queues


def _plan(edge_src: np.ndarray, edge_dst: np.ndarray):
    """Sort/bucket edges; build per-core int16-wrapped pair-idx + dstw arrays,
    unified call lengths and per-(call, window, parity) tile spans."""
    src = edge_src.astype(np.int64)
    dst = edge_dst.astype(np.int64)

    core = dst // NPC
    ldst = dst - core * NPC
    w = ldst // P
    sg = w // SGB
    b_local = w - sg * SGB
    chunk = src // CH
    sloc = src - chunk * CH
    # in-supergroup dst position
    dpos = 128 * b_local + (ldst - w * P)

    # sort edges by (core, sg, chunk, w)
    key = (((core * NSG + sg) * NCHUNK + chunk) * NWIN + w).astype(np.int64)
    order = np.argsort(key, kind="stable")
    key_s = key[order]
    pidx_s = sloc[order]
    dpos_s = dpos[order]

    nb = NWIN                                                 # window buckets
    ngroups = NCORES * NSG * NCHUNK * nb
    counts = np.bincount(key_s, minlength=ngroups).reshape(NCORES, NSG, NCHUNK, nb)
    call_core = counts.sum(axis=3)                            # [NCORES, NSG, NCHUNK]
    ucall = call_core.max(axis=0)
    ucall = ((ucall + P - 1) // P) * P
    ucall = np.maximum(ucall, P)

    call_base = np.zeros((NSG, NCHUNK), np.int64)
    pos = 0
    for s in range(NSG):
        for c in range(NCHUNK):
            call_base[s, c] = pos
            pos += ucall[s, c]
    cap_total = int(pos)
    cols = cap_total // P

    # per-core bucket starts within the call
    bstart = np.zeros((NCORES, NSG, NCHUNK, nb), np.int64)
    np.cumsum(counts[:, :, :, :-1], axis=3, out=bstart[:, :, :, 1:])
    bend = bstart + counts

    # per-edge slot position
    starts = np.zeros(ngroups + 1, np.int64)
    np.cumsum(np.bincount(key_s, minlength=ngroups), out=starts[1:])
    rank = np.arange(len(key_s)) - starts[key_s]
    core_s = key_s // (NSG * NCHUNK * nb)
    rem = key_s - core_s * (NSG * NCHUNK * nb)
    sg_s = rem // (NCHUNK * nb)
    rem2 = rem - sg_s * (NCHUNK * nb)
    c_s = rem2 // nb
    b_s = rem2 - c_s * nb
    pos_edge = call_base[sg_s, c_s] + bstart[core_s, sg_s, c_s, b_s] + rank

    slot_idx = np.full((NCORES, cap_total), CH, np.int16)       # pad -> zeros row
    slot_dpos = np.full((NCORES, cap_total), -1.0, np.float32)  # pad -> no window
    slot_idx[core_s, pos_edge] = pidx_s.astype(np.int16)
    slot_dpos[core_s, pos_edge] = dpos_s.astype(np.float32)

    # per-(sg, chunk, b_local) tile span within the call, core union
    tile_lo = np.zeros((NSG, NCHUNK, SGB), np.int64)
    tile_hi = np.zeros((NSG, NCHUNK, SGB), np.int64)
    for s in range(NSG):
        blo = s * SGB
        for c in range(NCHUNK):
            st = bstart[:, s, c, blo : blo + SGB]              # [NCORES, SGB]
            en = bend[:, s, c, blo : blo + SGB]
            ncols = int(ucall[s, c]) // P
            lo = np.minimum((st // P).min(axis=0), ncols - 1)
            hi = np.minimum(((en + P - 1) // P).max(axis=0), ncols)
            hi = np.maximum(hi, lo + 1)
            tile_lo[s, c] = lo
            tile_hi[s, c] = hi

    # int16 wrap: element i -> [i % 16, i // 16], replicated to 128 partitions
    idxw = slot_idx.reshape(NCORES, cap_total // 16, 16).transpose(0, 2, 1)
    idxw = np.tile(idxw, (1, 8, 1)).copy()                     # [NCORES, 128, cap/16]

    # dstw: element i -> [i % 128, i // 128]
    dstw = slot_dpos.reshape(NCORES, cols, P).transpose(0, 2, 1).astype(np.float16)

    return {
        "ucall": ucall,
        "call_base": call_base,
        "cols": cols,
        "tile_lo": tile_lo,
        "tile_hi": tile_hi,
        "idxw": idxw,
        "dstw": dstw,
    }


@with_exitstack
def _build(ctx: ExitStack, tc, plan, eps_scale: float):
    nc = tc.nc
    f32 = mybir.dt.float32
    bf16 = mybir.dt.float16

    ucall = plan["ucall"]
    call_base = plan["call_base"]
    cols = plan["cols"]
    tile_lo = plan["tile_lo"]
    tile_hi = plan["tile_hi"]

    tab = nc.dram_tensor("tab", [TAB_ROWS, 2 * D], bf16, kind="ExternalInput").ap()
    dstw_in = nc.dram_tensor("dstw", [P, cols], bf16, kind="ExternalInput").ap()
    x_in = nc.dram_tensor("x", [NSG * P, SGB * D], f32, kind="ExternalInput").ap()
    idxw_in = nc.dram_tensor("idxw", [P, cols * 8], mybir.dt.int16, kind="ExternalInput").ap()
    w1_in = nc.dram_tensor("w1", [D, DH], bf16, kind="ExternalInput").ap()
    b1_in = nc.dram_tensor("b1", [DH, 1], f32, kind="ExternalInput").ap()
    w2_in = nc.dram_tensor("w2", [DH, D], bf16, kind="ExternalInput").ap()
    b2_in = nc.dram_tensor("b2", [D, 1], f32, kind="ExternalInput").ap()
    ident_in = nc.dram_tensor("ident", [P, P], bf16, kind="ExternalInput").ap()
    iota_in = nc.dram_tensor("iota", [P, SGB * P], bf16, kind="ExternalInput").ap()
    out = nc.dram_tensor("out", [NSG * P, SGB * D], f32, kind="ExternalOutput").ap()

    consts = ctx.enter_context(tc.tile_pool(name="consts", bufs=1))
    xpool = ctx.enter_context(tc.tile_pool(name="xp", bufs=2))
    hxpool = ctx.enter_context(tc.tile_pool(name="hxp", bufs=2))
    obpool = ctx.enter_context(tc.tile_pool(name="obp", bufs=2))
    idxpool = ctx.enter_context(tc.tile_pool(name="idxp", bufs=6))
    dwpool = ctx.enter_context(tc.tile_pool(name="dwp", bufs=6))
    mpool = ctx.enter_context(tc.tile_pool(name="msgs", bufs=8))
    ohpool = ctx.enter_context(tc.tile_pool(name="oh", bufs=8))
    hpool = ctx.enter_context(tc.tile_pool(name="hp", bufs=3))
    sbp = ctx.enter_context(tc.tile_pool(name="sbp", bufs=3))
    psagg = ctx.enter_context(tc.tile_pool(name="psagg", bufs=2, space="PSUM"))
    pstp = ctx.enter_context(tc.tile_pool(name="pstp", bufs=1, space="PSUM"))
    psz1 = ctx.enter_context(tc.tile_pool(name="psz1", bufs=2, space="PSUM"))
    psoT = ctx.enter_context(tc.tile_pool(name="psoT", bufs=2, space="PSUM"))

    w1s = consts.tile([D, DH], bf16)
    nc.sync.dma_start(w1s[:], w1_in[:])
    w2s = consts.tile([DH, D], bf16)
    nc.sync.dma_start(w2s[:], w2_in[:])
    b1s = consts.tile([DH, 1], f32)
    nc.sync.dma_start(b1s[:], b1_in[:])
    b2s = consts.tile([D, 1], f32)
    nc.sync.dma_start(b2s[:], b2_in[:])
    ident = consts.tile([P, P], bf16)
    nc.sync.dma_start(ident[:], ident_in[:])
    # iota[:, i] = i; window b compares against [128*b, 128*b + 128)
    iota = consts.tile([P, SGB * P], bf16)
    nc.sync.dma_start(iota[:], iota_in[:])

    max_ccols = int(ucall.max()) // P
    qn = 0
    for s in range(NSG):
        # x slab for this supergroup, scaled by (1+eps)
        xs = xpool.tile([P, SGB * D], f32, tag="xs")
        nc.sync.dma_start(xs[:], x_in[s * P : (s + 1) * P, :])
        if eps_scale == 1.0:
            hx = xs
        else:
            hx = hxpool.tile([P, SGB * D], f32, tag="hx")
            nc.vector.tensor_scalar_mul(hx[:], xs[:], eps_scale)

        obsg = obpool.tile([P, SGB * D], f32, tag="obsg")
        msgs_c = []
        oh_c = []
        seg_off_c = []
        for c in range(NCHUNK):
            n_call = int(ucall[s, c])
            ccols = n_call // P
            idx_t = idxpool.tile([P, max(1, n_call // 16)], mybir.dt.int16, tag="idx")
            c16 = int(call_base[s, c]) // 16
            nc.sync.dma_start(idx_t[:], idxw_in[:, c16 : c16 + n_call // 16])
            dw_t = dwpool.tile([P, max_ccols], bf16, tag="dw")
            c128 = int(call_base[s, c]) // P
            nc.sync.dma_start(dw_t[:, :ccols], dstw_in[:, c128 : c128 + ccols])

            msgs = mpool.tile([P, max_ccols * 2 * D], bf16, tag="m")
            nc.gpsimd.dma_gather(
                out_ap=msgs[:, : ccols * 2 * D].rearrange(
                    "p (t f) -> p t f", t=ccols, f=2 * D
                ),
                in_ap=tab[c * (CH + 1) : (c + 1) * (CH + 1), :],
                idxs_ap=idx_t[:, : n_call // 16],
                num_idxs=n_call,
                num_idxs_reg=n_call,
                elem_size=2 * D,
                single_packet=False,
                queue_num=qn,
            )
            qn = (qn + 1) % NQ

            # one-hot segments, one per window bucket
            spans = [int(tile_hi[s, c, b] - tile_lo[s, c, b]) for b in range(SGB)]
            seg_off = []
            off = 0
            for b in range(SGB):
                seg_off.append(off)
                off += spans[b] * P
            oh = ohpool.tile([P, max(P, off)], mybir.dt.float8e4, tag="oh")
            for b in range(SGB):
                t_lo = int(tile_lo[s, c, b])
                span = spans[b]
                if span <= 0:
                    continue
                seg = oh[:, seg_off[b] : seg_off[b] + span * P]
                nc.vector.tensor_tensor(
                    out=seg.rearrange("p (t j) -> p t j", t=span, j=P),
                    in0=dw_t[:, t_lo : t_lo + span]
                    .rearrange("p (t o) -> p t o", o=1)
                    .to_broadcast([P, span, P]),
                    in1=iota[:, b * P : (b + 1) * P]
                    .rearrange("p (o j) -> p o j", o=1)
                    .to_broadcast([P, span, P]),
                    op=mybir.AluOpType.is_equal,
                )
            msgs_c.append(msgs)
            oh_c.append(oh)
            seg_off_c.append(seg_off)

        for b in range(SGB):
            blk = s * SGB + b
            total_mm = sum(
                int(tile_hi[s, c, b] - tile_lo[s, c, b]) for c in range(NCHUNK)
            )
            agg = psagg.tile([P, D], f32, tag="agg")
            done = 0
            for c in range(NCHUNK):
                t_lo = int(tile_lo[s, c, b])
                span = int(tile_hi[s, c, b] - t_lo)
                so = seg_off_c[c][b]
                for t in range(span):
                    col = t_lo + t
                    nc.tensor.matmul(
                        out=agg[:],
                        lhsT=oh_c[c][:, so + t * P : so + (t + 1) * P],
                        rhs=msgs_c[c][:, col * 2 * D : col * 2 * D + D],
                        start=(done == 0),
                        stop=(done == total_mm - 1),
                    )
                    done += 1

            h = hpool.tile([P, D], bf16, tag="h")
            nc.vector.tensor_add(h[:], agg[:], hx[:, b * D : (b + 1) * D])

            hT_ps = pstp.tile([D, P], bf16, tag="tp")
            nc.tensor.transpose(out=hT_ps[:], in_=h[:], identity=ident[:])
            hT = sbp.tile([D, P], bf16, tag="hT")
            nc.scalar.copy(hT[:], hT_ps[:])

            z1_ps = psz1.tile([DH, P], f32, tag="z1")
            nc.tensor.matmul(out=z1_ps[:], lhsT=w1s[:], rhs=hT[:], start=True, stop=True)
            z1r = sbp.tile([DH, P], bf16, tag="z1r")
            nc.scalar.activation(z1r[:], z1_ps[:], mybir.ActivationFunctionType.Relu, bias=b1s[:])

            oT_ps = psoT.tile([D, P], f32, tag="oT")
            nc.tensor.matmul(out=oT_ps[:], lhsT=w2s[:], rhs=z1r[:], start=True, stop=True)
            oT = sbp.tile([D, P], bf16, tag="oTs")
            nc.scalar.activation(oT[:], oT_ps[:], mybir.ActivationFunctionType.Identity, bias=b2s[:])

            ob_ps = pstp.tile([P, D], bf16, tag="tp2")
            nc.tensor.transpose(out=ob_ps[:], in_=oT[:], identity=ident[:D, :D])
            nc.scalar.copy(obsg[:, b * D : (b + 1) * D], ob_ps[:])
        nc.sync.dma_start(out[s * P : (s + 1) * P, :], obsg[:])


def kernel(node_attr, W1, b1, W2, b2, eps, edge_src, edge_dst):
    import ml_dtypes

    bf = np.float16
    node_attr = np.asarray(node_attr, np.float32)
    W1 = np.asarray(W1, np.float32)
    b1 = np.asarray(b1, np.float32)
    W2 = np.asarray(W2, np.float32)
    b2 = np.asarray(b2, np.float32)
    eps_scale = 1.0 + float(np.asarray(eps))
    edge_src = np.asarray(edge_src, np.int32)
    edge_dst = np.asarray(edge_dst, np.int32)

    plan = _plan(edge_src, edge_dst)

    # chunked fp16 table, rows padded to 256 B: [x[n] | zeros], zeros row per chunk
    tab = np.zeros((TAB_ROWS, 2 * D), bf)
    xb = node_attr.astype(bf)
    for c in range(NCHUNK):
        tab[c * (CH + 1) : c * (CH + 1) + CH, :D] = xb[c * CH : (c + 1) * CH]

    x_pad = np.zeros((NCORES * NPC, D), np.float32)
    x_pad[:N] = node_attr
    # per-core x, re-laid-out so each supergroup slab is a contiguous
    # [128, SGB*D] tile: row (s*128 + p) holds x[core_base + s*896 + b*128 + p]
    x_r = (
        x_pad.reshape(NCORES, NSG, SGB, P, D)
        .transpose(0, 1, 3, 2, 4)
        .reshape(NCORES, NSG * P, SGB * D)
    )

    ident = np.eye(P, dtype=bf)
    iota = np.tile(np.arange(SGB * P, dtype=np.float16), (P, 1)).copy()

    import concourse.bacc as bacc

    nc = bacc.Bacc(
        "TRN2",
        target_bir_lowering=False,
        debug=False,
        num_devices=NCORES,
        num_swdge_queues=NQ,
        dynamic_dma_scratch_size=32768,
    )
    with tile.TileContext(nc) as t:
        _build(t, plan, eps_scale)
    nc.compile()

    in_maps = []
    for c in range(NCORES):
        in_maps.append(
            {
                "tab": tab,
                "x": x_r[c],
                "idxw": plan["idxw"][c],
                "dstw": plan["dstw"][c],
                "w1": W1.astype(bf),
                "b1": b1.reshape(DH, 1),
                "w2": W2.astype(bf),
                "b2": b2.reshape(D, 1),
                "ident": ident,
                "iota": iota,
            }
        )

    res = bass_utils.run_bass_kernel_spmd(nc, in_maps, core_ids=list(range(NCORES)))
    import os as _os

    if res.exec_time_ns is not None:
        _os.environ["KERNEL_EXEC_NS"] = str(res.exec_time_ns)
    if res.instructions_and_trace is not None:
        print("trace:", res.instructions_and_trace[1])
    outs = []
    for r in res.results:
        o = (
            r["out"]
            .reshape(NSG, P, SGB, D)
            .transpose(0, 2, 1, 3)
            .reshape(NPC, D)
        )
        outs.append(o)
    full = np.concatenate(outs, axis=0)
    return full[:N]


if __name__ == "__main__":
    rng = np.random.default_rng(0)
    na = rng.normal(size=(N, D)).astype(np.float32)
    W1 = rng.normal(size=(D, DH)).astype(np.float32)
    b1 = np.zeros(DH, np.float32)
    W2 = rng.normal(size=(DH, D)).astype(np.float32)
    b2 = np.zeros(D, np.float32)
    eps = np.zeros((), np.float32)
    es = rng.integers(0, N, size=E).astype(np.int32)
    ed = rng.integers(0, N, size=E).astype(np.int32)
    out = kernel(na, W1, b1, W2, b2, eps, es, ed)
    print(out.shape, out.dtype)
